# revision 1
# baseline (speedup 1.0000x reference)
"""Trainium2 Bass kernel for nn_BidPrefix (segment_reduce).

Problem: inputs [B=500000, 302] f32 rows = [rates[0:300], market_price, bid].
  cp1[k] = prod(rates[:k])  (exclusive prefix products, cp1[0] = 1)
  survival  = cp1[bid]
  rate_last = cp1[mp] - cp1[mp+1]

Kernel strategy (pure data parallel over 8 NeuronCores, batch sharded):
  Per 128-row tile on each core:
    - one DMA loads the [128, 302] tile into SBUF;
    - DVE tensor_tensor_scan computes the exact sequential f32 cumprod
      into cp1[:, 1:301] (cp1[:, 0] preset to 1.0 once per physical
      buffer) — identical rounding to the reference's jnp.cumprod;
    - three DVE scalar_tensor_tensor instructions perform exact per-row
      gathers: accum_out = sum((iota == idx) * cp1) = cp1[idx]. The
      cp1[mp+1] gather uses a shifted cp1 view so no idx arithmetic is
      needed. Gather results land in per-tile columns of persistent
      [128, ntiles] accumulators.
  Tail: rate_last = A1 - A2 in one wide subtract, then two DMAs store
  the accumulators to DRAM ([ntiles*128, 1] interleaved layout).

The whole kernel is DVE-bound (4 full-width DVE passes per tile); DMA,
ACT, PE, GPSIMD stay under its shadow.
"""

import numpy as np

SEQ = 300
W = SEQ + 2  # input columns
CP = SEQ + 1  # cumprod-with-leading-one columns
B = 500000
N_CORES = 8
ROWS_PER_CORE = 62592  # 489 tiles of 128 rows; 8*62592 = 500736 >= B
assert ROWS_PER_CORE % 128 == 0 and N_CORES * ROWS_PER_CORE >= B

_CACHE = {}


def _split_multi_waits(nc, max_waits=1):
    """Walrus in this container rejects instructions with >1 sync-wait.

    Hoist extra waits onto single-wait NOPs inserted right before the
    offending instruction on the same engine (same-queue program order
    preserves semantics).
    """
    import concourse.mybir as mybir

    ctr = 0
    for fn in nc.m.functions:
        for bb in fn.blocks:
            il = bb.instructions
            i = 0
            while i < len(il):
                ins = il[i]
                si = ins.sync_info
                if si is not None and si.on_wait and len(si.on_wait) > max_waits:
                    waits = list(si.on_wait)
                    pos = i
                    for w in waits[max_waits:]:
                        ctr += 1
                        nop = mybir.InstNoOp(
                            name=f"I-splitwait-{ctr}",
                            engine=ins.engine,
                            sync_info=mybir.SyncInfo(on_wait=[w], on_update=[]),
                        )
                        il.insert(pos, nop)
                        pos += 1
                        i += 1
                    si.on_wait = waits[:max_waits]
                i += 1


def _build_nc(rows=ROWS_PER_CORE, n_cp_bufs=4, in_bufs=4, trash_bufs=4):
    import concourse.bass as bass
    import concourse.tile as tile
    from concourse import mybir

    F32 = mybir.dt.float32
    I32 = mybir.dt.int32
    assert rows % 128 == 0
    ntiles = rows // 128

    nc = bass.Bass("TRN2")
    x = nc.dram_tensor("inputs", [rows, W], F32, kind="ExternalInput")
    out_s = nc.dram_tensor("surv", [rows, 1], F32, kind="ExternalOutput")
    out_r = nc.dram_tensor("ratelast", [rows, 1], F32, kind="ExternalOutput")

    x_t = x.rearrange("(t p) c -> t p c", p=128)
    out_s_t = out_s.rearrange("(t p) c -> p t c", p=128)
    out_r_t = out_r.rearrange("(t p) c -> p t c", p=128)

    with tile.TileContext(nc) as tc:
        with (
            tc.tile_pool(name="inp", bufs=in_bufs) as inp_pool,
            tc.tile_pool(name="trash", bufs=trash_bufs) as trash_pool,
            tc.tile_pool(name="persist", bufs=1) as persist,
        ):
            S = persist.tile([128, ntiles], F32, tag="acc_s")
            A1 = persist.tile([128, ntiles], F32, tag="acc_a1")
            A2 = persist.tile([128, ntiles], F32, tag="acc_a2")

            iota_i = persist.tile([128, CP], I32, tag="iota_i")
            nc.gpsimd.iota(iota_i[:, :], [[1, CP]], channel_multiplier=0)
            iota_f = persist.tile([128, CP], F32, tag="iota_f")
            nc.vector.tensor_copy(iota_f[:, :], iota_i[:, :])

            cp1_bufs = []
            for i in range(n_cp_bufs):
                t = persist.tile([128, CP], F32, tag=f"cp1_{i}")
                nc.gpsimd.memset(t[:, 0:1], 1.0)
                cp1_bufs.append(t)

            for i in range(ntiles):
                xt = inp_pool.tile([128, W], F32, tag="xt")
                nc.sync.dma_start(out=xt[:, :], in_=x_t[i, :, :])

                rates = xt[:, 0:SEQ]
                mp = xt[:, SEQ : SEQ + 1]
                bid = xt[:, SEQ + 1 : SEQ + 2]

                cp1 = cp1_bufs[i % n_cp_bufs]
                nc.vector.tensor_tensor_scan(
                    out=cp1[:, 1:CP],
                    data0=rates,
                    data1=rates,
                    initial=1.0,
                    op0=mybir.AluOpType.mult,
                    op1=mybir.AluOpType.bypass,
                )

                for idx_ap, data_ap, iota_ap, acc in (
                    (bid, cp1[:, :], iota_f[:, :], S[:, i : i + 1]),
                    (mp, cp1[:, :], iota_f[:, :], A1[:, i : i + 1]),
                    (mp, cp1[:, 1:CP], iota_f[:, 0 : CP - 1], A2[:, i : i + 1]),
                ):
                    tr = trash_pool.tile([128, CP], F32, tag="tr")
                    nc.vector.scalar_tensor_tensor(
                        out=tr[:, 0 : iota_ap.shape[1]],
                        in0=iota_ap,
                        scalar=idx_ap,
                        in1=data_ap,
                        op0=mybir.AluOpType.is_equal,
                        op1=mybir.AluOpType.mult,
                        accum_out=acc,
                    )

            nc.vector.tensor_sub(A1[:, :], A1[:, :], A2[:, :])
            nc.sync.dma_start(out=out_s_t[:, :, 0], in_=S[:, :])
            nc.sync.dma_start(out=out_r_t[:, :, 0], in_=A1[:, :])

    _split_multi_waits(nc)
    return nc


def _get_nc():
    if "nc" not in _CACHE:
        _CACHE["nc"] = _build_nc()
    return _CACHE["nc"]


def _shard_inputs(inputs):
    total = N_CORES * ROWS_PER_CORE
    padded = np.empty((total, W), dtype=np.float32)
    padded[: inputs.shape[0]] = inputs
    if total > inputs.shape[0]:
        padded[inputs.shape[0] :, :SEQ] = 1.0
        padded[inputs.shape[0] :, SEQ:] = 0.0
    return [
        padded[c * ROWS_PER_CORE : (c + 1) * ROWS_PER_CORE] for c in range(N_CORES)
    ]


def kernel(inputs: np.ndarray):
    from concourse.bass_utils import run_bass_kernel_spmd

    inputs = np.ascontiguousarray(inputs, dtype=np.float32)
    assert inputs.shape == (B, W), inputs.shape

    nc = _get_nc()
    shards = _shard_inputs(inputs)
    res = run_bass_kernel_spmd(
        nc,
        [{"inputs": s} for s in shards],
        core_ids=list(range(N_CORES)),
    )
    surv = np.concatenate([r["surv"] for r in res.results], axis=0)[:B]
    rl = np.concatenate([r["ratelast"] for r in res.results], axis=0)[:B]
    return surv, rl

